# revision 7
# baseline (speedup 1.0000x reference)
"""Multi-head attention (B=4, G=2048, E=768, H=4) on 8 TRN2 NeuronCores.

Sharding: core c = (batch b = c//2, query-half qh = c%2). Each core computes
K/V for all 2048 tokens of its batch and attention + output projection for
its 1024 queries. The host rolls the token order per core so the local
queries are always tokens [0:1024] -> identical SPMD program on all cores,
no collectives (key order inside softmax sums is permutation-invariant).

Device dataflow (all heavy matmuls in float32r, fp32 PSUM accumulation):
  xT resident in SBUF (contraction dim on partitions).
  V phase:   Vext[h][tt] (128 tok, 192) = (x @ Wv + bv) per head/token-tile.
  QK phase:  KT/QT tiles (128 c, tokens) = (x @ Wqk + b)^T, c = (head,dim)
             packed naturally in 128-chunks.
  Attention: per (head, 512-query block): ET = K Q^T by 128-key tiles ->
             exp on ACT (scale 1/sqrt(768), no max subtraction: |E*s|<~4)
             -> avT(d,q) = V^T @ att accumulated over key tiles; row sums
             via ones-vector matmul; normalize by reciprocal broadcast
             (outer-product matmul).
  Proj:      out(q,e) = avs^T @ Wproj + bp, DMA per 128-query tile.
"""
import sys

sys.path.insert(0, "/opt/trn_rl_repo")
sys.path.insert(0, "/root/.axon_site")

from contextlib import ExitStack

import numpy as np

import concourse.bass as bass
import concourse.tile as tile
from concourse import bacc, mybir
from concourse.bass_utils import run_bass_kernel_spmd

N_CORES = 8
B, G, E, H = 4, 2048, 768, 4
D = E // H            # 192
HALF = G // 2         # 1024 queries per core
KCH = E // 128        # 6 contraction chunks
SCALE = 1.0 / float(np.sqrt(E))

f32 = mybir.dt.float32
f32r = mybir.dt.float32r


def _c_chunks(h):
    """Split head h's c-range [h*192,(h+1)*192) on 128-tile boundaries.

    Returns (tile_idx, partition_offset, length); offsets are always 0 or 64.
    """
    out = []
    c, c1 = h * D, (h + 1) * D
    while c < c1:
        ti, off = divmod(c, 128)
        ln = min(128 - off, c1 - c)
        out.append((ti, off, ln))
        c += ln
    return out


def _emit(nc, t):
    with ExitStack() as top:
        tc = top.enter_context(tile.TileContext(nc))
        const = top.enter_context(tc.tile_pool(name="const", bufs=1))
        kqt_p = top.enter_context(tc.tile_pool(name="kqt", bufs=1))
        v_p = top.enter_context(tc.tile_pool(name="vext", bufs=1))

        ones1 = const.tile([1, 128], f32, tag="ones1")
        nc.vector.memset(ones1[:], 1.0)
        onesK = const.tile([128, 1], f32r, tag="onesK")
        nc.sync.dma_start(onesK[:], t["onesk"][:].bitcast(f32r))
        bqk_sb = const.tile([128, 12], f32, tag="bqk")
        nc.sync.dma_start(bqk_sb[:], t["bqk"][:])
        bv_sb = const.tile([1, E], f32, tag="bv")
        nc.sync.dma_start(bv_sb[:], t["bv"][:])
        bp_sb = const.tile([1, E], f32, tag="bp")
        nc.sync.dma_start(bp_sb[:], t["bp"][:])
        bv_bc = const.tile([128, E], f32, tag="bv_bc")
        bp_bc = const.tile([128, E], f32, tag="bp_bc")

        kt_sb = [kqt_p.tile([128, G], f32r, tag=f"kt{i}", name=f"kt{i}")
                 for i in range(6)]
        qt_sb = [kqt_p.tile([128, HALF], f32r, tag=f"qt{i}", name=f"qt{i}")
                 for i in range(6)]
        vext = [[v_p.tile([128, D + 1], f32r, tag=f"v{h}_{tt}", name=f"v{h}_{tt}")
                 for tt in range(16)] for h in range(H)]

        with tc.tile_pool(name="xt_pool", bufs=1) as xt_p:
            xt = xt_p.tile([128, KCH * G], f32r, tag="xt")
            nc.sync.dma_start(xt[:], t["xt"][:].bitcast(f32r))

            # ---- V phase (+ broadcast biases along partitions via matmul) --
            with tc.tile_pool(name="vps", bufs=2, space="PSUM") as vps, \
                 tc.tile_pool(name="wv_pool", bufs=1) as wvp:
                for j in range(2):
                    bb = vps.tile([128, 384], f32, tag="va")
                    nc.tensor.matmul(bb[:], ones1[:], bv_sb[:, j * 384:(j + 1) * 384],
                                     start=True, stop=True)
                    nc.vector.tensor_copy(bv_bc[:, j * 384:(j + 1) * 384], bb[:])
                    bb2 = vps.tile([128, 384], f32, tag="vb")
                    nc.tensor.matmul(bb2[:], ones1[:], bp_sb[:, j * 384:(j + 1) * 384],
                                     start=True, stop=True)
                    nc.vector.tensor_copy(bp_bc[:, j * 384:(j + 1) * 384], bb2[:])

                wv_sb = wvp.tile([128, KCH * E], f32r, tag="wv")
                nc.sync.dma_start(wv_sb[:], t["wv"][:].bitcast(f32r))
                for tt in range(16):
                    pa = vps.tile([128, 384], f32, tag="va")
                    pb = vps.tile([128, 384], f32, tag="vb")
                    for k in range(KCH):
                        lhsT = xt[:, k * G + tt * 128: k * G + tt * 128 + 128]
                        nc.tensor.matmul(pa[:], lhsT, wv_sb[:, k * E: k * E + 384],
                                         start=(k == 0), stop=(k == KCH - 1))
                        nc.tensor.matmul(pb[:], lhsT, wv_sb[:, k * E + 384: k * E + 768],
                                         start=(k == 0), stop=(k == KCH - 1))
                    for h in range(H):
                        src = pa if h < 2 else pb
                        col = (h % 2) * D
                        nc.vector.tensor_add(vext[h][tt][:, 0:D], src[:, col:col + D],
                                             bv_bc[:, h * D: h * D + D])
                        nc.vector.tensor_copy(vext[h][tt][:, D:D + 1], onesK[:])

            # ---- QK phase -------------------------------------------------
            with tc.tile_pool(name="qkps", bufs=4, space="PSUM") as qkps, \
                 tc.tile_pool(name="wqk_pool", bufs=3) as wqkp:
                for tblk in range(12):
                    wt = wqkp.tile([128, KCH * 128], f32r, tag="wqk")
                    nc.sync.dma_start(
                        wt[:], t["wqk"][:, tblk * 768:(tblk + 1) * 768].bitcast(f32r))
                    is_k = tblk < 6
                    dest = kt_sb[tblk] if is_k else qt_sb[tblk - 6]
                    for n in range(4 if is_k else 2):
                        ps = qkps.tile([128, 512], f32, tag="qk")
                        tok0 = n * 512
                        for k in range(KCH):
                            nc.tensor.matmul(
                                ps[:], wt[:, k * 128:(k + 1) * 128],
                                xt[:, k * G + tok0: k * G + tok0 + 512],
                                start=(k == 0), stop=(k == KCH - 1))
                        nc.vector.tensor_scalar_add(
                            dest[:, tok0:tok0 + 512], ps[:],
                            bqk_sb[:, tblk:tblk + 1])

        # ---- attention + projection (xt freed) ---------------------------
        with tc.tile_pool(name="etps", bufs=2, space="PSUM") as et_ps, \
             tc.tile_pool(name="avps", bufs=2, space="PSUM") as av_ps, \
             tc.tile_pool(name="pps", bufs=1, space="PSUM") as p_ps, \
             tc.tile_pool(name="att_pool", bufs=2) as att_p, \
             tc.tile_pool(name="avs_pool", bufs=2) as avs_p, \
             tc.tile_pool(name="r_pool", bufs=1) as r_p, \
             tc.tile_pool(name="out_pool", bufs=1) as out_p, \
             tc.tile_pool(name="wp_pool", bufs=1) as wpp:
            wp_sb = wpp.tile([96, 8 * E], f32r, tag="wp")
            nc.sync.dma_start(wp_sb[:], t["wp"][:].bitcast(f32r))

            for qb in range(2):
                avs_tiles = []
                for h in range(H):
                    avT0 = av_ps.tile([96, 512], f32, tag="avT0")
                    avT1 = av_ps.tile([97, 512], f32, tag="avT1")
                    for kc in range(16):
                        et = et_ps.tile([128, 512], f32, tag="et")
                        chunks = _c_chunks(h)
                        for ci, (ti, off, ln) in enumerate(chunks):
                            nc.tensor.matmul(
                                et[:],
                                kt_sb[ti][off:off + ln, kc * 128:(kc + 1) * 128],
                                qt_sb[ti][off:off + ln, qb * 512:(qb + 1) * 512],
                                start=(ci == 0), stop=(ci == len(chunks) - 1))
                        att = att_p.tile([128, 512], f32r, tag="att")
                        nc.scalar.activation(
                            att[:], et[:], mybir.ActivationFunctionType.Exp,
                            scale=SCALE)
                        vt = vext[h][kc]
                        nc.tensor.matmul(avT0[:], vt[:, 0:96], att[:],
                                         start=(kc == 0), stop=(kc == 15))
                        # cols 96..192 = V d96..191, col 192 = ones -> row 96
                        # of avT1 accumulates the softmax denominators
                        nc.tensor.matmul(avT1[:], vt[:, 96:193], att[:],
                                         start=(kc == 0), stop=(kc == 15))
                    s96 = r_p.tile([97, 512], f32, tag="s96")
                    nc.vector.tensor_copy(s96[96:97, :], avT1[96:97, :])
                    r0 = r_p.tile([1, 512], f32, tag="r0")
                    nc.sync.dma_start(r0[:], s96[96:97, :])
                    rr = r_p.tile([1, 512], f32, tag="rr")
                    nc.vector.reciprocal_approx_fast(rr[:], r0[:])
                    bc = et_ps.tile([128, 512], f32, tag="et")
                    nc.tensor.matmul(bc[:], ones1[:], rr[:], start=True, stop=True)
                    bc_sb = r_p.tile([128, 512], f32, tag="bcsb")
                    nc.vector.tensor_copy(bc_sb[:], bc[:])
                    for dc, avT in enumerate((avT0, avT1)):
                        avs = avs_p.tile([96, 512], f32r, tag=f"avs{h}_{dc}")
                        nc.vector.tensor_mul(avs[:], avT[0:96, :], bc_sb[0:96, :])
                        avs_tiles.append(avs)

                for qs in range(4):
                    p0 = p_ps.tile([128, 384], f32, tag="p0")
                    p1 = p_ps.tile([128, 384], f32, tag="p1")
                    for cc in range(8):
                        lhsT = avs_tiles[cc][:, qs * 128:(qs + 1) * 128]
                        nc.tensor.matmul(p0[:], lhsT, wp_sb[:, cc * 768: cc * 768 + 384],
                                         start=(cc == 0), stop=(cc == 7))
                        nc.tensor.matmul(p1[:], lhsT, wp_sb[:, cc * 768 + 384: cc * 768 + 768],
                                         start=(cc == 0), stop=(cc == 7))
                    osb = out_p.tile([128, E], f32, tag="osb")
                    nc.vector.tensor_add(osb[:, 0:384], p0[:], bp_bc[:, 0:384])
                    nc.vector.tensor_add(osb[:, 384:768], p1[:], bp_bc[:, 384:768])
                    row = qb * 512 + qs * 128
                    nc.sync.dma_start(t["y"][row:row + 128, :], osb[:])


_CACHED_NC = None


def _get_nc():
    global _CACHED_NC
    if _CACHED_NC is None:
        nc = bacc.Bacc("TRN2", target_bir_lowering=False, debug=False,
                       num_devices=N_CORES)
        t = {
            "xt": nc.dram_tensor("xt", (128, KCH * G), f32, kind="ExternalInput").ap(),
            "wqk": nc.dram_tensor("wqk", (128, 12 * 768), f32, kind="ExternalInput").ap(),
            "wv": nc.dram_tensor("wv", (128, KCH * E), f32, kind="ExternalInput").ap(),
            "wp": nc.dram_tensor("wp", (96, 8 * E), f32, kind="ExternalInput").ap(),
            "bqk": nc.dram_tensor("bqk", (128, 12), f32, kind="ExternalInput").ap(),
            "bv": nc.dram_tensor("bv", (1, E), f32, kind="ExternalInput").ap(),
            "bp": nc.dram_tensor("bp", (1, E), f32, kind="ExternalInput").ap(),
            "onesk": nc.dram_tensor("onesk", (128, 1), f32, kind="ExternalInput").ap(),
            "y": nc.dram_tensor("y", (HALF, E), f32, kind="ExternalOutput").ap(),
        }
        _emit(nc, t)
        nc.compile()
        _CACHED_NC = nc
    return _CACHED_NC


def _pack_contraction(w, rows=128):
    """(R, C) -> (rows, R//rows * C): contraction chunks on partitions,
    per-partition data contiguous (k-major along free dim)."""
    r, c = w.shape
    n = r // rows
    return np.ascontiguousarray(
        w.reshape(n, rows, c).transpose(1, 0, 2).reshape(rows, n * c))


def make_in_maps(x, W_qkv, b_qkv, W_proj, b_proj):
    x = np.asarray(x, dtype=np.float32)
    W_qkv = np.asarray(W_qkv, dtype=np.float32)
    b_qkv = np.asarray(b_qkv, dtype=np.float32)
    W_proj = np.asarray(W_proj, dtype=np.float32)
    b_proj = np.asarray(b_proj, dtype=np.float32)

    # qkv column factorization: col = (h, d, {q,k,v}) with qkv fastest
    Wf = W_qkv.reshape(E, H, D, 3)
    bf = b_qkv.reshape(H, D, 3)
    Wq = np.ascontiguousarray(Wf[..., 0].reshape(E, E))
    Wk = np.ascontiguousarray(Wf[..., 1].reshape(E, E))
    Wv = np.ascontiguousarray(Wf[..., 2].reshape(E, E))
    bq = np.ascontiguousarray(bf[..., 0].reshape(E))
    bk = np.ascontiguousarray(bf[..., 1].reshape(E))
    bv = np.ascontiguousarray(bf[..., 2].reshape(E))

    # 12 column blocks of 128: first 6 = K, then 6 = Q; each packed k-major
    blocks = [_pack_contraction(Wk[:, i * 128:(i + 1) * 128]) for i in range(6)]
    blocks += [_pack_contraction(Wq[:, i * 128:(i + 1) * 128]) for i in range(6)]
    wqk = np.concatenate(blocks, axis=1)  # (128, 12*768)
    bqk = np.stack([bk[i * 128:(i + 1) * 128] for i in range(6)]
                   + [bq[i * 128:(i + 1) * 128] for i in range(6)], axis=1)  # (128,12)

    wv_packed = _pack_contraction(Wv)  # (128, 4608)
    # W_proj rows c=(h,d) split into 8 chunks of 96 on partitions
    wp = np.ascontiguousarray(
        W_proj.reshape(8, 96, E).transpose(1, 0, 2).reshape(96, 8 * E))

    shared = {
        "wqk": wqk, "wv": wv_packed, "wp": wp, "bqk": bqk,
        "bv": bv.reshape(1, E), "bp": b_proj.reshape(1, E).astype(np.float32),
        "onesk": np.ones((128, 1), dtype=np.float32),
    }
    in_maps = []
    for c in range(N_CORES):
        b, qh = divmod(c, 2)
        # roll tokens so the local query half is first (key-order invariant)
        xb = x[b] if qh == 0 else np.concatenate([x[b, HALF:], x[b, :HALF]], axis=0)
        xt = _pack_contraction(np.ascontiguousarray(xb.T))  # (128, 6*2048)
        in_maps.append({"xt": xt, **shared})
    return in_maps


def kernel(**inputs):
    nc = _get_nc()
    in_maps = make_in_maps(inputs["x"], inputs["W_qkv"], inputs["b_qkv"],
                           inputs["W_proj"], inputs["b_proj"])
    res = run_bass_kernel_spmd(nc, in_maps, core_ids=list(range(N_CORES)))
    out = np.empty((B, G, E), dtype=np.float32)
    for c in range(N_CORES):
        b, qh = divmod(c, 2)
        out[b, qh * HALF:(qh + 1) * HALF, :] = res.results[c]["y"]
    return out


# revision 8
# speedup vs baseline: 1.0240x; 1.0240x over previous
"""Multi-head attention (B=4, G=2048, E=768, H=4) on 8 TRN2 NeuronCores.

Sharding: core c = (batch b = c//2, query-half qh = c%2). Each core computes
K/V for all 2048 tokens of its batch and attention + output projection for
its 1024 queries. The host rolls the token order per core so the local
queries are always tokens [0:1024] -> identical SPMD program on all cores,
no collectives (key order inside softmax sums is permutation-invariant).

Device dataflow (all heavy matmuls in float32r, fp32 PSUM accumulation):
  xT resident in SBUF (contraction dim on partitions).
  V phase:   Vext[h][tt] (128 tok, 192) = (x @ Wv + bv) per head/token-tile.
  QK phase:  KT/QT tiles (128 c, tokens) = (x @ Wqk + b)^T, c = (head,dim)
             packed naturally in 128-chunks.
  Attention: per (head, 512-query block): ET = K Q^T by 128-key tiles ->
             exp on ACT (scale 1/sqrt(768), no max subtraction: |E*s|<~4)
             -> avT(d,q) = V^T @ att accumulated over key tiles; row sums
             via ones-vector matmul; normalize by reciprocal broadcast
             (outer-product matmul).
  Proj:      out(q,e) = avs^T @ Wproj + bp, DMA per 128-query tile.
"""
import sys

sys.path.insert(0, "/opt/trn_rl_repo")
sys.path.insert(0, "/root/.axon_site")

from contextlib import ExitStack

import numpy as np

import concourse.bass as bass
import concourse.tile as tile
from concourse import bacc, mybir
from concourse.bass_utils import run_bass_kernel_spmd

N_CORES = 8
B, G, E, H = 4, 2048, 768, 4
D = E // H            # 192
HALF = G // 2         # 1024 queries per core
KCH = E // 128        # 6 contraction chunks
SCALE = 1.0 / float(np.sqrt(E))

f32 = mybir.dt.float32
f32r = mybir.dt.float32r


def _c_chunks(h):
    """Split head h's c-range [h*192,(h+1)*192) on 128-tile boundaries.

    Returns (tile_idx, partition_offset, length); offsets are always 0 or 64.
    """
    out = []
    c, c1 = h * D, (h + 1) * D
    while c < c1:
        ti, off = divmod(c, 128)
        ln = min(128 - off, c1 - c)
        out.append((ti, off, ln))
        c += ln
    return out


def _emit(nc, t):
    with ExitStack() as top:
        tc = top.enter_context(tile.TileContext(nc))
        const = top.enter_context(tc.tile_pool(name="const", bufs=1))
        kqt_p = top.enter_context(tc.tile_pool(name="kqt", bufs=1))
        v_p = top.enter_context(tc.tile_pool(name="vext", bufs=1))

        ones1 = const.tile([1, 128], f32, tag="ones1")
        nc.vector.memset(ones1[:], 1.0)
        onesK = const.tile([128, 1], f32r, tag="onesK")
        nc.sync.dma_start(onesK[:], t["onesk"][:].bitcast(f32r))
        bqk_sb = const.tile([128, 12], f32, tag="bqk")
        nc.sync.dma_start(bqk_sb[:], t["bqk"][:])
        bv_sb = const.tile([1, E], f32, tag="bv")
        nc.sync.dma_start(bv_sb[:], t["bv"][:])
        bp_sb = const.tile([1, E], f32, tag="bp")
        nc.sync.dma_start(bp_sb[:], t["bp"][:])
        bv_bc = const.tile([128, E], f32, tag="bv_bc")
        bp_bc = const.tile([128, E], f32, tag="bp_bc")

        kt_sb = [kqt_p.tile([128, G], f32r, tag=f"kt{i}", name=f"kt{i}")
                 for i in range(6)]
        qt_sb = [kqt_p.tile([128, HALF], f32r, tag=f"qt{i}", name=f"qt{i}")
                 for i in range(6)]
        vext = [[v_p.tile([128, D + 1], f32r, tag=f"v{h}_{tt}", name=f"v{h}_{tt}")
                 for tt in range(16)] for h in range(H)]

        with tc.tile_pool(name="xt_pool", bufs=1) as xt_p:
            xt = xt_p.tile([128, KCH * G], f32r, tag="xt")
            for k in range(KCH):
                nc.sync.dma_start(xt[:, k * G:(k + 1) * G],
                                  t["xt"][:, k * G:(k + 1) * G].bitcast(f32r))

            # ---- V phase (+ broadcast biases along partitions via matmul) --
            with tc.tile_pool(name="vps", bufs=2, space="PSUM") as vps, \
                 tc.tile_pool(name="wv_pool", bufs=1) as wvp:
                for j in range(2):
                    bb = vps.tile([128, 384], f32, tag="va")
                    nc.tensor.matmul(bb[:], ones1[:], bv_sb[:, j * 384:(j + 1) * 384],
                                     start=True, stop=True)
                    nc.vector.tensor_copy(bv_bc[:, j * 384:(j + 1) * 384], bb[:])
                    bb2 = vps.tile([128, 384], f32, tag="vb")
                    nc.tensor.matmul(bb2[:], ones1[:], bp_sb[:, j * 384:(j + 1) * 384],
                                     start=True, stop=True)
                    nc.vector.tensor_copy(bp_bc[:, j * 384:(j + 1) * 384], bb2[:])

                wv_sb = wvp.tile([128, KCH * E], f32r, tag="wv")
                nc.sync.dma_start(wv_sb[:], t["wv"][:].bitcast(f32r))
                for tt in range(16):
                    pa = vps.tile([128, 384], f32, tag="va")
                    pb = vps.tile([128, 384], f32, tag="vb")
                    for k in range(KCH):
                        lhsT = xt[:, k * G + tt * 128: k * G + tt * 128 + 128]
                        nc.tensor.matmul(pa[:], lhsT, wv_sb[:, k * E: k * E + 384],
                                         start=(k == 0), stop=(k == KCH - 1))
                        nc.tensor.matmul(pb[:], lhsT, wv_sb[:, k * E + 384: k * E + 768],
                                         start=(k == 0), stop=(k == KCH - 1))
                    for h in range(H):
                        src = pa if h < 2 else pb
                        col = (h % 2) * D
                        nc.vector.tensor_add(vext[h][tt][:, 0:D], src[:, col:col + D],
                                             bv_bc[:, h * D: h * D + D])
                        nc.vector.tensor_copy(vext[h][tt][:, D:D + 1], onesK[:])

            # ---- QK phase -------------------------------------------------
            with tc.tile_pool(name="qkps", bufs=4, space="PSUM") as qkps, \
                 tc.tile_pool(name="wqk_pool", bufs=3) as wqkp:
                for tblk in range(12):
                    wt = wqkp.tile([128, KCH * 128], f32r, tag="wqk")
                    nc.sync.dma_start(
                        wt[:], t["wqk"][:, tblk * 768:(tblk + 1) * 768].bitcast(f32r))
                    is_k = tblk < 6
                    dest = kt_sb[tblk] if is_k else qt_sb[tblk - 6]
                    for n in range(4 if is_k else 2):
                        ps = qkps.tile([128, 512], f32, tag="qk")
                        tok0 = n * 512
                        for k in range(KCH):
                            nc.tensor.matmul(
                                ps[:], wt[:, k * 128:(k + 1) * 128],
                                xt[:, k * G + tok0: k * G + tok0 + 512],
                                start=(k == 0), stop=(k == KCH - 1))
                        nc.vector.tensor_scalar_add(
                            dest[:, tok0:tok0 + 512], ps[:],
                            bqk_sb[:, tblk:tblk + 1])

        # ---- attention + projection (xt freed) ---------------------------
        with tc.tile_pool(name="etps", bufs=3, space="PSUM") as et_ps, \
             tc.tile_pool(name="avps", bufs=2, space="PSUM") as av_ps, \
             tc.tile_pool(name="att_pool", bufs=2) as att_p, \
             tc.tile_pool(name="avs_pool", bufs=2) as avs_p, \
             tc.tile_pool(name="r_pool", bufs=1) as r_p, \
             tc.tile_pool(name="out_pool", bufs=1) as out_p, \
             tc.tile_pool(name="wp_pool", bufs=1) as wpp:
            wp_sb = wpp.tile([96, 8 * E], f32r, tag="wp")
            nc.sync.dma_start(wp_sb[:], t["wp"][:].bitcast(f32r))

            for qb in range(2):
                avs_tiles = []
                for h in range(H):
                    avT0 = av_ps.tile([96, 512], f32, tag="avT0")
                    avT1 = av_ps.tile([97, 512], f32, tag="avT1")
                    for kc in range(16):
                        et = et_ps.tile([128, 512], f32, tag="et")
                        chunks = _c_chunks(h)
                        for ci, (ti, off, ln) in enumerate(chunks):
                            nc.tensor.matmul(
                                et[:],
                                kt_sb[ti][off:off + ln, kc * 128:(kc + 1) * 128],
                                qt_sb[ti][off:off + ln, qb * 512:(qb + 1) * 512],
                                start=(ci == 0), stop=(ci == len(chunks) - 1))
                        att = att_p.tile([128, 512], f32r, tag="att")
                        nc.scalar.activation(
                            att[:], et[:], mybir.ActivationFunctionType.Exp,
                            scale=SCALE)
                        vt = vext[h][kc]
                        nc.tensor.matmul(avT0[:], vt[:, 0:96], att[:],
                                         start=(kc == 0), stop=(kc == 15))
                        # cols 96..192 = V d96..191, col 192 = ones -> row 96
                        # of avT1 accumulates the softmax denominators
                        nc.tensor.matmul(avT1[:], vt[:, 96:193], att[:],
                                         start=(kc == 0), stop=(kc == 15))
                    s96 = r_p.tile([97, 512], f32, tag="s96")
                    nc.vector.tensor_copy(s96[96:97, :], avT1[96:97, :])
                    r0 = r_p.tile([1, 512], f32, tag="r0")
                    nc.sync.dma_start(r0[:], s96[96:97, :])
                    rr = r_p.tile([1, 512], f32, tag="rr")
                    nc.vector.reciprocal_approx_fast(rr[:], r0[:])
                    bc = et_ps.tile([128, 512], f32, tag="et")
                    nc.tensor.matmul(bc[:], ones1[:], rr[:], start=True, stop=True)
                    bc_sb = r_p.tile([128, 512], f32, tag="bcsb")
                    nc.vector.tensor_copy(bc_sb[:], bc[:])
                    for dc, avT in enumerate((avT0, avT1)):
                        avs = avs_p.tile([96, 512], f32r, tag=f"avs{h}_{dc}")
                        nc.vector.tensor_mul(avs[:], avT[0:96, :], bc_sb[0:96, :])
                        avs_tiles.append(avs)

                for qs in range(4):
                    p0 = et_ps.tile([128, 384], f32, tag="et", name="p0")
                    p1 = et_ps.tile([128, 384], f32, tag="et", name="p1")
                    for cc in range(8):
                        lhsT = avs_tiles[cc][:, qs * 128:(qs + 1) * 128]
                        nc.tensor.matmul(p0[:], lhsT, wp_sb[:, cc * 768: cc * 768 + 384],
                                         start=(cc == 0), stop=(cc == 7))
                        nc.tensor.matmul(p1[:], lhsT, wp_sb[:, cc * 768 + 384: cc * 768 + 768],
                                         start=(cc == 0), stop=(cc == 7))
                    osb = out_p.tile([128, E], f32, tag="osb")
                    nc.vector.tensor_add(osb[:, 0:384], p0[:], bp_bc[:, 0:384])
                    nc.vector.tensor_add(osb[:, 384:768], p1[:], bp_bc[:, 384:768])
                    row = qb * 512 + qs * 128
                    nc.sync.dma_start(t["y"][row:row + 128, :], osb[:])


_CACHED_NC = None


def _get_nc():
    global _CACHED_NC
    if _CACHED_NC is None:
        nc = bacc.Bacc("TRN2", target_bir_lowering=False, debug=False,
                       num_devices=N_CORES)
        t = {
            "xt": nc.dram_tensor("xt", (128, KCH * G), f32, kind="ExternalInput").ap(),
            "wqk": nc.dram_tensor("wqk", (128, 12 * 768), f32, kind="ExternalInput").ap(),
            "wv": nc.dram_tensor("wv", (128, KCH * E), f32, kind="ExternalInput").ap(),
            "wp": nc.dram_tensor("wp", (96, 8 * E), f32, kind="ExternalInput").ap(),
            "bqk": nc.dram_tensor("bqk", (128, 12), f32, kind="ExternalInput").ap(),
            "bv": nc.dram_tensor("bv", (1, E), f32, kind="ExternalInput").ap(),
            "bp": nc.dram_tensor("bp", (1, E), f32, kind="ExternalInput").ap(),
            "onesk": nc.dram_tensor("onesk", (128, 1), f32, kind="ExternalInput").ap(),
            "y": nc.dram_tensor("y", (HALF, E), f32, kind="ExternalOutput").ap(),
        }
        _emit(nc, t)
        nc.compile()
        _CACHED_NC = nc
    return _CACHED_NC


def _pack_contraction(w, rows=128):
    """(R, C) -> (rows, R//rows * C): contraction chunks on partitions,
    per-partition data contiguous (k-major along free dim)."""
    r, c = w.shape
    n = r // rows
    return np.ascontiguousarray(
        w.reshape(n, rows, c).transpose(1, 0, 2).reshape(rows, n * c))


def make_in_maps(x, W_qkv, b_qkv, W_proj, b_proj):
    x = np.asarray(x, dtype=np.float32)
    W_qkv = np.asarray(W_qkv, dtype=np.float32)
    b_qkv = np.asarray(b_qkv, dtype=np.float32)
    W_proj = np.asarray(W_proj, dtype=np.float32)
    b_proj = np.asarray(b_proj, dtype=np.float32)

    # qkv column factorization: col = (h, d, {q,k,v}) with qkv fastest
    Wf = W_qkv.reshape(E, H, D, 3)
    bf = b_qkv.reshape(H, D, 3)
    Wq = np.ascontiguousarray(Wf[..., 0].reshape(E, E))
    Wk = np.ascontiguousarray(Wf[..., 1].reshape(E, E))
    Wv = np.ascontiguousarray(Wf[..., 2].reshape(E, E))
    bq = np.ascontiguousarray(bf[..., 0].reshape(E))
    bk = np.ascontiguousarray(bf[..., 1].reshape(E))
    bv = np.ascontiguousarray(bf[..., 2].reshape(E))

    # 12 column blocks of 128: first 6 = K, then 6 = Q; each packed k-major
    blocks = [_pack_contraction(Wk[:, i * 128:(i + 1) * 128]) for i in range(6)]
    blocks += [_pack_contraction(Wq[:, i * 128:(i + 1) * 128]) for i in range(6)]
    wqk = np.concatenate(blocks, axis=1)  # (128, 12*768)
    bqk = np.stack([bk[i * 128:(i + 1) * 128] for i in range(6)]
                   + [bq[i * 128:(i + 1) * 128] for i in range(6)], axis=1)  # (128,12)

    wv_packed = _pack_contraction(Wv)  # (128, 4608)
    # W_proj rows c=(h,d) split into 8 chunks of 96 on partitions
    wp = np.ascontiguousarray(
        W_proj.reshape(8, 96, E).transpose(1, 0, 2).reshape(96, 8 * E))

    shared = {
        "wqk": wqk, "wv": wv_packed, "wp": wp, "bqk": bqk,
        "bv": bv.reshape(1, E), "bp": b_proj.reshape(1, E).astype(np.float32),
        "onesk": np.ones((128, 1), dtype=np.float32),
    }
    in_maps = []
    for c in range(N_CORES):
        b, qh = divmod(c, 2)
        # roll tokens so the local query half is first (key-order invariant)
        xb = x[b] if qh == 0 else np.concatenate([x[b, HALF:], x[b, :HALF]], axis=0)
        xt = _pack_contraction(np.ascontiguousarray(xb.T))  # (128, 6*2048)
        in_maps.append({"xt": xt, **shared})
    return in_maps


def kernel(**inputs):
    nc = _get_nc()
    in_maps = make_in_maps(inputs["x"], inputs["W_qkv"], inputs["b_qkv"],
                           inputs["W_proj"], inputs["b_proj"])
    res = run_bass_kernel_spmd(nc, in_maps, core_ids=list(range(N_CORES)))
    out = np.empty((B, G, E), dtype=np.float32)
    for c in range(N_CORES):
        b, qh = divmod(c, 2)
        out[b, qh * HALF:(qh + 1) * HALF, :] = res.results[c]["y"]
    return out
